# revision 1
# baseline (speedup 1.0000x reference)
"""Trainium2 Bass kernel for the CN coupling-block problem (nn_CN_69312182223156).

Math (per subnet s on half-features x_s with conditioner c):
    h   = relu(c @ W1 + b1)                       # [B, 50]
    p   = h @ W2 + b2                             # [B, 9696]
    m1, b1p, m2 = p[:, :3200], p[:, 3200:6400], p[:, 6400:9600]   (viewed [B,32,100])
    bias2, eps, alpha = p[:, 9600:9632], p[:, 9632:9664]/10, p[:, 9664:]/10
    z   = x*m1 + b1p
    num = sum_l elu(z)*m2 ;  den = sum_l relu(-m1*m2) + 1
    y   = exp(alpha) * (x + 0.8*sigmoid(eps)*num/den) + bias2

Subnet 1: x=x1, c=x2.  Subnet 2: x=x2, c=y1.  Output concat([y1, y2]).

Strategy: pure data-parallel over 8 cores (2048 rows each), weights replicated.
Layout: batch on SBUF partitions (tiles of 128 rows). All matmuls on PE with
biases folded in via augmented weights (extra ones-row/column), including
S2 = sum_l mat2 as 32 extra output columns so `num` needs no -1 term:
    elu(z)+1 = exp(min(z,0)) + relu(z)
    num = sum_l (elu(z)+1)*m2 - S2 ;  den = sum_l relu(-m1*m2) + 1
The [B, 9696] intermediate is produced into paired PSUM banks in 800-column
chunks and consumed immediately: ScalarE does one PSUM->SBUF f16 cast per
stream chunk plus exp and relu(-m1*m2); VectorE does the per-dim x-broadcast
tensor_scalar (4x rate), the f16 tensor_tensor products (2x rate), and a
folded reduction (two 2x-rate pair-adds, then a 1x tensor_reduce over 25).
Issue order is phase-split (all subnet-1 tiles, then all subnet-2 tiles) so
the scheduler always has independent work around the y1 dependency.
Cost-model exec time: ~631 us/core; both vector engines ~93% busy.
"""

import numpy as np

B = 16384
DIM = 32
LS = 100
NCORES = 8
BC = B // NCORES          # rows per core
NT = BC // 128            # 128-row tiles per core
DL = DIM * LS             # 3200
PW = 3 * DL + 3 * DIM     # 9696 params per row
CHUNK = 800               # params per elementwise chunk (8 dims x 100)
HALF = 400                # params per PSUM-bank matmul
NCHUNK = DL // CHUNK      # 4
DPC = CHUNK // LS         # 8 dims per chunk

_cache = {}


def _build_program():
    import concourse.bass as bass
    import concourse.tile as tile
    import concourse.mybir as mybir
    from concourse import bacc, masks

    f32 = mybir.dt.float32
    f16 = mybir.dt.float16
    Alu = mybir.AluOpType
    Act = mybir.ActivationFunctionType
    X = mybir.AxisListType.X

    nc = bacc.Bacc("TRN2", target_bir_lowering=False)

    x_d = nc.dram_tensor("x", [BC, 2 * DIM], f32, kind="ExternalInput")
    w1a = [nc.dram_tensor(f"w1a{s}", [DIM + 1, 51], f16, kind="ExternalInput")
           for s in (1, 2)]
    w2a = [nc.dram_tensor(f"w2a{s}", [51, PW + DIM], f16, kind="ExternalInput")
           for s in (1, 2)]
    y_d = nc.dram_tensor("y", [BC, 2 * DIM], f32, kind="ExternalOutput")

    with tile.TileContext(nc) as tc:
        with (
            tc.tile_pool(name="const", bufs=1) as const,
            tc.tile_pool(name="io", bufs=4) as io,
            tc.tile_pool(name="mid", bufs=4) as mid,
            tc.tile_pool(name="ew", bufs=6) as ew,
            tc.tile_pool(name="tail", bufs=3) as tailp,
            tc.tile_pool(name="per", bufs=1) as per,
            tc.tile_pool(name="pmm", bufs=3, space="PSUM") as pmm,
            tc.tile_pool(name="psm", bufs=2, space="PSUM") as psm,
        ):
            # ---- constants ----
            w1s = []
            w2s = []
            for s in range(2):
                t1 = const.tile([DIM + 1, 51], f16, tag=f"w1_{s}")
                nc.sync.dma_start(t1, w1a[s][:])
                w1s.append(t1)
                t2 = const.tile([51, PW + DIM], f16, tag=f"w2_{s}")
                nc.sync.dma_start(t2, w2a[s][:])
                w2s.append(t2)
            ident = const.tile([128, 128], f16, tag="ident")
            masks.make_identity(nc, ident[:])
            identf = const.tile([128, 128], f32, tag="identf")
            masks.make_identity(nc, identf[:])
            negone = const.tile([128, 1], f32, tag="negone")
            nc.vector.memset(negone, -1.0)

            def subnet(s, it, xf, condT, y_out):
                # h^T = relu(W1^T c^T + b1): [51, 128]; col 50 of W1aug is
                # e_32 so row 50 comes out as relu(1) = 1 (the aug ones row).
                h_ps = psm.tile([51, 128], f32, tag="tp")
                nc.tensor.matmul(h_ps, w1s[s], condT, start=True, stop=True)
                hT = mid.tile([51, 128], f16, tag="hT")
                nc.scalar.activation(hT, h_ps, Act.Relu)

                xc32 = xf[:, s * DIM:(s + 1) * DIM]   # f32 x for this subnet
                numden = ew.tile([128, 2, DIM], f32, tag="numden")

                for c in range(NCHUNK):
                    co = c * CHUNK
                    m1s = ew.tile([128, CHUNK], f16, tag="m1s")
                    b1s = ew.tile([128, CHUNK], f16, tag="b1s")
                    m2s = ew.tile([128, CHUNK], f16, tag="m2s")
                    for (dst, base) in ((m1s, 0), (b1s, DL), (m2s, 2 * DL)):
                        mp = pmm.tile([128, 2, 512], f32, tag="mm2")
                        for hh in range(CHUNK // HALF):
                            o = base + co + hh * HALF
                            nc.tensor.matmul(mp[:, hh, 0:HALF], hT,
                                             w2s[s][:, o:o + HALF],
                                             start=True, stop=True)
                        dst2 = dst.rearrange("p (h q) -> p h q", h=2)
                        nc.scalar.copy(dst2, mp[:, :, 0:HALF])

                    # z = x*m1 + b1  (per-dim tensor_scalar for the x broadcast)
                    zmul = ew.tile([128, CHUNK], f16, tag="zmul")
                    zm3 = zmul.rearrange("p (d l) -> p d l", l=LS)
                    m1s3 = m1s.rearrange("p (d l) -> p d l", l=LS)
                    for j in range(DPC):
                        nc.vector.tensor_scalar_mul(
                            zm3[:, j, :], m1s3[:, j, :],
                            xc32[:, c * DPC + j:c * DPC + j + 1])
                    # b1s carries b1+1, so z1 = z+1 and, using e^x >= 1+x:
                    #   elu(z)+1 = max(z+1, exp(min(z,0)))
                    z1 = ew.tile([128, CHUNK], f16, tag="z1")
                    nc.vector.tensor_add(z1, zmul, b1s)
                    zn = ew.tile([128, CHUNK], f16, tag="zn")
                    nc.vector.tensor_scalar_min(zn, z1, 1.0)
                    e = ew.tile([128, CHUNK], f16, tag="e")
                    nc.scalar.activation(e, zn, Act.Exp, bias=negone)
                    w = ew.tile([128, CHUNK], f16, tag="w")
                    nc.vector.tensor_tensor(w, z1, e, Alu.max)
                    tr = ew.tile([128, 2, CHUNK], f16, tag="tr")
                    # t = w*m2 = (elu(z)+1)*m2; sum_l m2 (S2) subtracted in tail
                    nc.vector.tensor_mul(tr[:, 0, :], w, m2s)
                    # u = m1*m2 ; r = relu(-u) on ACT
                    u = ew.tile([128, CHUNK], f16, tag="u")
                    nc.vector.tensor_mul(u, m1s, m2s)
                    if (it * NCHUNK + c) % 8 == 0:
                        nc.vector.tensor_scalar(tr[:, 1, :], u, -1.0, 0.0,
                                                Alu.mult, Alu.max)
                    else:
                        nc.scalar.activation(tr[:, 1, :], u, Act.Relu, scale=-1.0)
                    # two folding passes (2x-rate TT adds), then a 1x reduce
                    tr4 = tr.rearrange("p t (d f l) -> p t d f l", f=2, l=LS // 2)
                    th = ew.tile([128, 2, DPC, LS // 2], f16, tag="th")
                    nc.vector.tensor_add(th, tr4[:, :, :, 0, :], tr4[:, :, :, 1, :])
                    th4 = th.rearrange("p t d (f l) -> p t d f l", f=2)
                    th2 = ew.tile([128, 2, DPC, LS // 4], f16, tag="th2")
                    nc.vector.tensor_add(th2, th4[:, :, :, 0, :], th4[:, :, :, 1, :])
                    nc.vector.tensor_reduce(
                        numden[:, :, c * DPC:(c + 1) * DPC], th2, X, Alu.add)

                # ---- tail (bias2 | eps | alpha | S2) ----
                tp = psm.tile([128, 4 * DIM], f32, tag="tp")
                nc.tensor.matmul(tp, hT, w2s[s][:, 3 * DL:3 * DL + 4 * DIM],
                                 start=True, stop=True)
                b2p = tp[:, 0:DIM]
                epp = tp[:, DIM:2 * DIM]
                alp = tp[:, 2 * DIM:3 * DIM]
                s2p = tp[:, 3 * DIM:4 * DIM]

                den = tailp.tile([128, DIM], f32, tag="den")
                nc.vector.tensor_scalar_add(den, numden[:, 1, :], 1.0)
                rec = tailp.tile([128, DIM], f32, tag="rec")
                nc.vector.reciprocal_approx_fast(rec, den)
                # sigmoid(eps/10) = 1 / (1 + exp(-eps/10))
                nege = tailp.tile([128, DIM], f32, tag="nege")
                nc.scalar.activation(nege, epp, Act.Exp, scale=-0.1)
                sd = tailp.tile([128, DIM], f32, tag="sd")
                nc.vector.tensor_scalar_add(sd, nege, 1.0)
                sig = tailp.tile([128, DIM], f32, tag="sig")
                nc.vector.reciprocal_approx_fast(sig, sd)
                ea = tailp.tile([128, DIM], f32, tag="ea")
                nc.scalar.activation(ea, alp, Act.Exp, scale=0.1)
                nums = tailp.tile([128, DIM], f32, tag="nums")
                nc.vector.tensor_sub(nums, numden[:, 0, :], s2p)
                frac = tailp.tile([128, DIM], f32, tag="frac")
                nc.vector.tensor_mul(frac, nums, rec)
                q = tailp.tile([128, DIM], f32, tag="q")
                nc.vector.scalar_tensor_tensor(
                    q, in0=frac, scalar=0.8, in1=sig, op0=Alu.mult, op1=Alu.mult)
                sx = tailp.tile([128, DIM], f32, tag="sx")
                nc.vector.tensor_add(sx, q, xc32)
                yp = tailp.tile([128, DIM], f32, tag="yp")
                nc.vector.tensor_mul(yp, ea, sx)
                nc.vector.tensor_add(y_out[:, s * DIM:(s + 1) * DIM], yp, b2p)

            xfs, youts = {}, {}
            for it in range(NT):
                r0 = it * 128
                xf = per.tile([128, 2 * DIM + 1], f32, tag=f"xf{it}")
                nc.sync.dma_start(xf[:, 0:2 * DIM], x_d[r0:r0 + 128, :])
                nc.vector.memset(xf[:, 2 * DIM:], 1.0)

                # conditioner for subnet 1: [x2 | 1]^T  -> [33, 128]
                ct_ps = psm.tile([DIM + 1, 128], f32, tag="tp")
                nc.tensor.transpose(ct_ps, xf[:, DIM:2 * DIM + 1], identf)
                condT = mid.tile([DIM + 1, 128], f16, tag="condT")
                nc.scalar.copy(condT, ct_ps)

                y_out = per.tile([128, 2 * DIM], f32, tag=f"y_out{it}")
                subnet(0, it, xf, condT, y_out)
                xfs[it], youts[it] = xf, y_out

            for it in range(NT):
                r0 = it * 128
                xf, y_out = xfs[it], youts[it]
                # conditioner for subnet 2: [y1 | 1]^T
                c2_ps = psm.tile([DIM, 128], f32, tag="tp")
                nc.tensor.transpose(c2_ps, y_out[:, 0:DIM], identf)
                condT2 = mid.tile([DIM + 1, 128], f16, tag="condT2")
                nc.scalar.copy(condT2[0:DIM, :], c2_ps)
                nc.vector.memset(condT2[DIM:DIM + 1, :], 1.0)
                subnet(1, it, xf, condT2, y_out)
                nc.sync.dma_start(y_d[r0:r0 + 128, :], y_out)

    nc.compile()
    return nc


def _prep_weights(W1, b1, W2, b2):
    w1a = np.concatenate([W1, b1[None, :]], axis=0).astype(np.float16)  # [33, 50]
    ones_col = np.zeros((DIM + 1, 1), dtype=np.float16)
    ones_col[DIM, 0] = 1.0
    w1a = np.concatenate([w1a, ones_col], axis=1)                       # [33, 51]
    w2a = np.concatenate([W2, b2[None, :]], axis=0)                     # [51, 9696] f32
    w2a = w2a.copy()
    w2a[50, DL:2 * DL] += 1.0   # bias1 region delivers b1+1 (see w = max(z+1, e))
    # append S2 columns: S2[:, d] = sum_l w2a[:, mat2 region (d, l)]
    m2cols = w2a[:, 2 * DL:3 * DL].reshape(51, DIM, LS)
    s2 = m2cols.sum(axis=2)                                             # [51, DIM]
    w2a = np.concatenate([w2a, s2], axis=1).astype(np.float16)          # [51, 9728]
    return np.ascontiguousarray(w1a), np.ascontiguousarray(w2a)


def kernel(**inputs):
    from concourse.bass_utils import run_bass_kernel_spmd

    if "nc" not in _cache:
        _cache["nc"] = _build_program()
    nc = _cache["nc"]

    x = np.ascontiguousarray(inputs["x"], dtype=np.float32)
    w1a1, w2a1 = _prep_weights(inputs["s1_W1"], inputs["s1_b1"],
                               inputs["s1_W2"], inputs["s1_b2"])
    w1a2, w2a2 = _prep_weights(inputs["s2_W1"], inputs["s2_b1"],
                               inputs["s2_W2"], inputs["s2_b2"])

    in_maps = []
    for i in range(NCORES):
        in_maps.append({
            "x": x[i * BC:(i + 1) * BC],
            "w1a1": w1a1, "w2a1": w2a1,
            "w1a2": w1a2, "w2a2": w2a2,
        })

    last_err = None
    for attempt in range(3):
        try:
            res = run_bass_kernel_spmd(nc, in_maps, core_ids=list(range(NCORES)),
                                       **_cache.get("run_kwargs", {}))
            out = np.concatenate([r["y"] for r in res.results], axis=0)
            _cache["last_results"] = res
            return out
        except Exception as ex:  # transient NRT/device errors: retry
            last_err = ex
    raise last_err



# revision 6
# speedup vs baseline: 1.0506x; 1.0506x over previous
"""Trainium2 Bass kernel for the CN coupling-block problem (nn_CN_69312182223156).

Math (per subnet s on half-features x_s with conditioner c):
    h   = relu(c @ W1 + b1)                       # [B, 50]
    p   = h @ W2 + b2                             # [B, 9696]
    m1, b1p, m2 = p[:, :3200], p[:, 3200:6400], p[:, 6400:9600]   (viewed [B,32,100])
    bias2, eps, alpha = p[:, 9600:9632], p[:, 9632:9664]/10, p[:, 9664:]/10
    z   = x*m1 + b1p
    num = sum_l elu(z)*m2 ;  den = sum_l relu(-m1*m2) + 1
    y   = exp(alpha) * (x + 0.8*sigmoid(eps)*num/den) + bias2

Subnet 1: x=x1, c=x2.  Subnet 2: x=x2, c=y1.  Output concat([y1, y2]).

Strategy: pure data-parallel over 8 cores (2048 rows each), weights replicated.
Layout: batch on SBUF partitions (tiles of 128 rows); the [B, 9696] parameter
tensor is produced on PE in 800-column chunks (8 dims x 100) and consumed
immediately.  Three-engine balance (vs. two in the earlier version):
  - ACT: one merged strided PSUM->SBUF f16 cast for m1+b1 (one 1600-col op),
    one for m2, plus the exp.
  - DVE: per-dim x-broadcast tensor_scalar (4x rate), z1 add, min, w*m2
    product, two pair-fold adds + reduce for num, and the den reduction as
    8 per-dim tensor_scalar(mult,-1 / max,0) ops with fused accum_out
    (replaces relu + fold chain for den).
  - GPSIMD (idle before): w = max(z1, e) via tensor_max and u = m1*m2 via
    tensor_tensor mult (only TT-class SBUF ops are supported by the
    compiler on Pool).
Biases fold into augmented weights; S2 = sum_l mat2 rides as 32 extra matmul
columns so num needs no -1 term:  elu(z)+1 = max(z+1, exp(min(z,0))).
Tails (bias2|eps|alpha|S2 + num/den combine) are batched over groups of 4
row-tiles to amortize instruction overheads.
"""

import numpy as np

B = 16384
DIM = 32
LS = 100
NCORES = 8
BC = B // NCORES          # rows per core
NT = BC // 128            # 128-row tiles per core
GT = 4                    # tiles per tail group
DL = DIM * LS             # 3200
PW = 3 * DL + 3 * DIM     # 9696 params per row
CHUNK = 800               # params per elementwise chunk (8 dims x 100)
HALF = 400                # params per PSUM-bank matmul
NCHUNK = DL // CHUNK      # 4
DPC = CHUNK // LS         # 8 dims per chunk

_cache = {}


def _build_program():
    import concourse.bass as bass
    import concourse.tile as tile
    import concourse.mybir as mybir
    from concourse import bacc, masks

    f32 = mybir.dt.float32
    f16 = mybir.dt.float16
    Alu = mybir.AluOpType
    Act = mybir.ActivationFunctionType
    X = mybir.AxisListType.X

    nc = bacc.Bacc("TRN2", target_bir_lowering=False)

    x_d = nc.dram_tensor("x", [BC, 2 * DIM], f32, kind="ExternalInput")
    w1a = [nc.dram_tensor(f"w1a{s}", [DIM + 1, 51], f16, kind="ExternalInput")
           for s in (1, 2)]
    w2a = [nc.dram_tensor(f"w2a{s}", [51, PW + DIM], f16, kind="ExternalInput")
           for s in (1, 2)]
    y_d = nc.dram_tensor("y", [BC, 2 * DIM], f32, kind="ExternalOutput")

    with tile.TileContext(nc) as tc:
        with (
            tc.tile_pool(name="const", bufs=1) as const,
            tc.tile_pool(name="per", bufs=1) as per,
            tc.tile_pool(name="mid", bufs=4) as mid,
            tc.tile_pool(name="ew", bufs=4) as ew,
            tc.tile_pool(name="tailp", bufs=2) as tailp,
            tc.tile_pool(name="pmm1", bufs=1, space="PSUM") as pmm1,
            tc.tile_pool(name="pmm2", bufs=1, space="PSUM") as pmm2,
            tc.tile_pool(name="psm", bufs=2, space="PSUM") as psm,
        ):
            # ---- constants ----
            w1s = []
            w2s = []
            for s in range(2):
                t1 = const.tile([DIM + 1, 51], f16, tag=f"w1_{s}", name="t1")
                nc.sync.dma_start(t1, w1a[s][:])
                w1s.append(t1)
                t2 = const.tile([51, PW + DIM], f16, tag=f"w2_{s}", name="t2")
                nc.sync.dma_start(t2, w2a[s][:])
                w2s.append(t2)
            identf = const.tile([128, 128], f32, tag="identf", name="identf")
            masks.make_identity(nc, identf[:])
            negone = const.tile([128, 1], f32, tag="negone", name="negone")
            nc.vector.memset(negone, -1.0)

            def subnet(s, it, xf, condT, nd4, itg):
                """One 128-row tile through subnet s.  Accumulates num/den
                into nd4[:, itg, :, :] ([128, 2, DIM] slice of the group
                tile); tail runs separately, batched per group."""
                # h^T = relu(W1^T c^T + b1): [51, 128]; col 50 of W1aug is
                # e_32 so row 50 comes out as relu(1) = 1 (the aug ones row).
                h_ps = psm.tile([51, 128], f32, tag="tp", name="h_ps")
                nc.tensor.matmul(h_ps, w1s[s], condT, start=True, stop=True)
                hT = mid.tile([51, 128], f16, tag="hT", name="hT")
                nc.scalar.activation(hT, h_ps, Act.Relu)

                xc32 = xf[:, s * DIM:(s + 1) * DIM]   # f32 x for this subnet

                for c in range(NCHUNK):
                    co = c * CHUNK
                    # -- PE: m1+b1 into one 4-bank tile, m2 into a 2-bank one
                    p1 = pmm1.tile([128, 4, 512], f32, tag="p1", name="p1")
                    p2 = pmm2.tile([128, 2, 512], f32, tag="p2", name="p2")
                    for hh in range(2):
                        o = co + hh * HALF
                        nc.tensor.matmul(p1[:, hh, 0:HALF], hT,
                                         w2s[s][:, o:o + HALF],
                                         start=True, stop=True)
                        nc.tensor.matmul(p1[:, 2 + hh, 0:HALF], hT,
                                         w2s[s][:, DL + o:DL + o + HALF],
                                         start=True, stop=True)
                        nc.tensor.matmul(p2[:, hh, 0:HALF], hT,
                                         w2s[s][:, 2 * DL + o:2 * DL + o + HALF],
                                         start=True, stop=True)
                    # -- ACT: one merged f16 cast for m1+b1, one for m2
                    mb = ew.tile([128, 4, HALF], f16, tag="mb", name="mb")
                    nc.scalar.copy(mb, p1[:, :, 0:HALF])
                    m2s = ew.tile([128, 2, HALF], f16, tag="m2s", name="m2s")
                    nc.scalar.copy(m2s, p2[:, :, 0:HALF])
                    m1f = mb[:, 0:2, :].rearrange("p h q -> p (h q)")
                    b1f = mb[:, 2:4, :].rearrange("p h q -> p (h q)")
                    m2f = m2s.rearrange("p h q -> p (h q)")

                    # z = x*m1 + b1  (per-dim tensor_scalar for the x broadcast)
                    zmul = ew.tile([128, CHUNK], f16, tag="zmul", name="zmul")
                    zm3 = zmul.rearrange("p (d l) -> p d l", l=LS)
                    m1s3 = m1f.rearrange("p (d l) -> p d l", l=LS)
                    for j in range(DPC):
                        nc.vector.tensor_scalar_mul(
                            zm3[:, j, :], m1s3[:, j, :],
                            xc32[:, c * DPC + j:c * DPC + j + 1])
                    # b1f carries b1+1, so z1 = z+1 and, using e^x >= 1+x:
                    #   elu(z)+1 = max(z+1, exp(min(z,0)))
                    z1 = ew.tile([128, CHUNK], f16, tag="z1", name="z1")
                    nc.vector.tensor_add(z1, zmul, b1f)
                    zn = ew.tile([128, CHUNK], f16, tag="zn", name="zn")
                    nc.vector.tensor_scalar_min(zn, z1, 1.0)
                    e = ew.tile([128, CHUNK], f16, tag="e", name="e")
                    nc.scalar.activation(e, zn, Act.Exp, bias=negone)
                    # u = m1*m2 on the (otherwise idle) Pool engine
                    w = ew.tile([128, CHUNK], f16, tag="w", name="w")
                    nc.vector.tensor_tensor(w, z1, e, Alu.max)
                    u = ew.tile([128, CHUNK], f16, tag="u", name="u")
                    nc.gpsimd.tensor_mul(u, m1f, m2f)
                    # num: t = w*m2 = (elu(z)+1)*m2 (S2 subtracted in tail),
                    # two folding passes (on Pool) then a reduce over 25.
                    tr0 = ew.tile([128, DPC, LS], f16, tag="tr0", name="tr0")
                    nc.vector.tensor_mul(tr0, w.rearrange("p (d l) -> p d l", l=LS), m2f.rearrange("p (d l) -> p d l", l=LS))
                    tr4 = tr0.rearrange("p d (f l) -> p d f l", f=2)
                    th = ew.tile([128, DPC, LS // 2], f16, tag="th", name="th")
                    nc.gpsimd.tensor_add(th, tr4[:, :, 0, :], tr4[:, :, 1, :])
                    th4 = th.rearrange("p d (f l) -> p d f l", f=2)
                    th2 = ew.tile([128, DPC, LS // 4], f16, tag="th2", name="th2")
                    nc.gpsimd.tensor_add(th2, th4[:, :, 0, :], th4[:, :, 1, :])
                    numo = nd4[:, itg, 0, c * DPC:(c + 1) * DPC]
                    nc.vector.tensor_reduce(numo, th2, X, Alu.add)
                    # den: per-dim relu(-u) with fused accumulate
                    scr = ew.tile([128, DPC, LS], f16, tag="scr", name="scr")
                    u3 = u.rearrange("p (d l) -> p d l", l=LS)
                    for j in range(DPC):
                        dd = c * DPC + j
                        nc.vector.tensor_scalar(
                            scr[:, j, :], u3[:, j, :], -1.0, 0.0,
                            Alu.mult, Alu.max,
                            accum_out=nd4[:, itg, 1, dd:dd + 1])
                return hT

            def tail_group(s, g, nd4, hTs, xfs, youts):
                """Batched tail for GT tiles: tail matmuls into one PSUM bank,
                elementwise at FD=128 on [128, GT, DIM] views."""
                tp4 = psm.tile([128, 4, 128], f32, tag="tp", name="tp4")
                for t in range(GT):
                    nc.tensor.matmul(tp4[:, t, :], hTs[t],
                                     w2s[s][:, 3 * DL:3 * DL + 4 * DIM],
                                     start=True, stop=True)
                b2p = tp4[:, :, 0:DIM]
                epp = tp4[:, :, DIM:2 * DIM]
                alp = tp4[:, :, 2 * DIM:3 * DIM]
                s2p = tp4[:, :, 3 * DIM:4 * DIM]
                num4 = nd4[:, :, 0, :]
                den4 = nd4[:, :, 1, :]

                den = tailp.tile([128, GT, DIM], f32, tag="den", name="den")
                nc.vector.tensor_scalar_add(den, den4, 1.0)
                rec = tailp.tile([128, GT, DIM], f32, tag="rec", name="rec")
                nc.vector.reciprocal_approx_fast(rec, den)
                # sigmoid(eps/10) = 1 / (1 + exp(-eps/10))
                nege = tailp.tile([128, GT, DIM], f32, tag="nege", name="nege")
                nc.scalar.activation(nege, epp, Act.Exp, scale=-0.1)
                sd = tailp.tile([128, GT, DIM], f32, tag="sd", name="sd")
                nc.vector.tensor_scalar_add(sd, nege, 1.0)
                sig = tailp.tile([128, GT, DIM], f32, tag="sig", name="sig")
                nc.vector.reciprocal_approx_fast(sig, sd)
                ea = tailp.tile([128, GT, DIM], f32, tag="ea", name="ea")
                nc.scalar.activation(ea, alp, Act.Exp, scale=0.1)
                nums = tailp.tile([128, GT, DIM], f32, tag="nums", name="nums")
                nc.vector.tensor_sub(nums, num4, s2p)
                frac = tailp.tile([128, GT, DIM], f32, tag="frac", name="frac")
                nc.vector.tensor_mul(frac, nums, rec)
                q = tailp.tile([128, GT, DIM], f32, tag="q", name="q")
                nc.vector.scalar_tensor_tensor(
                    q, in0=frac, scalar=0.8, in1=sig, op0=Alu.mult, op1=Alu.mult)
                # y = ea*(x+q) + b2; sx per tile since x lives per-tile
                for t in range(GT):
                    it = g * GT + t
                    xf, y_out = xfs[it], youts[it]
                    xc = xf[:, s * DIM:(s + 1) * DIM]
                    sx = tailp.tile([128, DIM], f32, tag="sx", name="sx")
                    nc.vector.tensor_add(sx, q[:, t, :], xc)
                    yp = tailp.tile([128, DIM], f32, tag="yp", name="yp")
                    nc.vector.tensor_mul(yp, ea[:, t, :], sx)
                    nc.vector.tensor_add(y_out[:, s * DIM:(s + 1) * DIM],
                                         yp, b2p[:, t, :])

            xfs, youts = {}, {}
            # ---------------- phase 1: subnet 1 on all tiles ----------------
            hTs = []
            nd4s = {}
            for it in range(NT):
                r0 = it * 128
                g, itg = it // GT, it % GT
                if itg == 0:
                    nd4s[(0, g)] = per.tile([128, GT, 2, DIM], f32,
                                            tag=f"nd0_{g}", name="nd4")
                xf = per.tile([128, 2 * DIM + 1], f32, tag=f"xf{it}", name="xf")
                nc.sync.dma_start(xf[:, 0:2 * DIM], x_d[r0:r0 + 128, :])
                nc.vector.memset(xf[:, 2 * DIM:], 1.0)

                # conditioner for subnet 1: [x2 | 1]^T  -> [33, 128]
                ct_ps = psm.tile([DIM + 1, 128], f32, tag="tp", name="ct_ps")
                nc.tensor.transpose(ct_ps, xf[:, DIM:2 * DIM + 1], identf)
                condT = mid.tile([DIM + 1, 128], f16, tag="condT", name="condT")
                nc.scalar.copy(condT, ct_ps)

                y_out = per.tile([128, 2 * DIM], f32, tag=f"y_out{it}", name="y_out")
                hT = subnet(0, it, xf, condT, nd4s[(0, g)], itg)
                hTs.append(hT)
                xfs[it], youts[it] = xf, y_out
                if itg == GT - 1:
                    tail_group(0, g, nd4s[(0, g)], hTs[-GT:], xfs, youts)

            # ---------------- phase 2: subnet 2 on all tiles ----------------
            hTs2 = []
            for it in range(NT):
                r0 = it * 128
                g, itg = it // GT, it % GT
                if itg == 0:
                    nd4s[(1, g)] = per.tile([128, GT, 2, DIM], f32,
                                            tag=f"nd1_{g}", name="nd4")
                xf, y_out = xfs[it], youts[it]
                # conditioner for subnet 2: [y1 | 1]^T
                c2_ps = psm.tile([DIM, 128], f32, tag="tp", name="c2_ps")
                nc.tensor.transpose(c2_ps, y_out[:, 0:DIM], identf)
                condT2 = mid.tile([DIM + 1, 128], f16, tag="condT", name="condT2")
                nc.scalar.copy(condT2[0:DIM, :], c2_ps)
                nc.vector.memset(condT2[DIM:DIM + 1, :], 1.0)
                hT = subnet(1, it, xf, condT2, nd4s[(1, g)], itg)
                hTs2.append(hT)
                if itg == GT - 1:
                    tail_group(1, g, nd4s[(1, g)], hTs2[-GT:], xfs, youts)
                    for t in range(GT):
                        it2 = g * GT + t
                        nc.sync.dma_start(y_d[it2 * 128:(it2 + 1) * 128, :],
                                          youts[it2])

    nc.compile()
    return nc


def _prep_weights(W1, b1, W2, b2):
    w1a = np.concatenate([W1, b1[None, :]], axis=0).astype(np.float16)  # [33, 50]
    ones_col = np.zeros((DIM + 1, 1), dtype=np.float16)
    ones_col[DIM, 0] = 1.0
    w1a = np.concatenate([w1a, ones_col], axis=1)                       # [33, 51]
    w2a = np.concatenate([W2, b2[None, :]], axis=0)                     # [51, 9696] f32
    w2a = w2a.copy()
    w2a[50, DL:2 * DL] += 1.0   # bias1 region delivers b1+1 (see w = max(z+1, e))
    # append S2 columns: S2[:, d] = sum_l w2a[:, mat2 region (d, l)]
    m2cols = w2a[:, 2 * DL:3 * DL].reshape(51, DIM, LS)
    s2 = m2cols.sum(axis=2)                                             # [51, DIM]
    w2a = np.concatenate([w2a, s2], axis=1).astype(np.float16)          # [51, 9728]
    return np.ascontiguousarray(w1a), np.ascontiguousarray(w2a)


def kernel(**inputs):
    from concourse.bass_utils import run_bass_kernel_spmd

    if "nc" not in _cache:
        _cache["nc"] = _build_program()
    nc = _cache["nc"]

    x = np.ascontiguousarray(inputs["x"], dtype=np.float32)
    w1a1, w2a1 = _prep_weights(inputs["s1_W1"], inputs["s1_b1"],
                               inputs["s1_W2"], inputs["s1_b2"])
    w1a2, w2a2 = _prep_weights(inputs["s2_W1"], inputs["s2_b1"],
                               inputs["s2_W2"], inputs["s2_b2"])

    in_maps = []
    for i in range(NCORES):
        in_maps.append({
            "x": x[i * BC:(i + 1) * BC],
            "w1a1": w1a1, "w2a1": w2a1,
            "w1a2": w1a2, "w2a2": w2a2,
        })

    last_err = None
    for attempt in range(3):
        try:
            res = run_bass_kernel_spmd(nc, in_maps, core_ids=list(range(NCORES)),
                                       **_cache.get("run_kwargs", {}))
            out = np.concatenate([r["y"] for r in res.results], axis=0)
            _cache["last_results"] = res
            return out
        except Exception as ex:  # transient NRT/device errors: retry
            last_err = ex
    raise last_err
